# revision 1
# baseline (speedup 1.0000x reference)
"""Causal self-attention (B=4, T=2048, C=2048, H=16) on 8 trn2 NeuronCores.

Sharding: tensor-parallel over heads — 2 heads per core. Every core gets the
full (pre-transposed) activation xT, its 2 heads' slice of Wqkv columns and
Wproj rows, computes a full [B*T, C] partial output, and the host sums the 8
partials (the "all-reduce after output projection" done host-side).

Per-core dataflow (all matmuls on PE, fp32r for x/W precision, fp16 for the
attention-probability path):
  xT tiles --DMA--> QKV proj -> Q^T,K^T [d,t] (f32r) + V [t,d] (fp16)
  S = Q^T.T @ K^T chunks (PSUM f32) -> +causal mask -> exp (ACT, accum denom)
  P (fp16) -> normalize by 1/denom (DVE) -> PE-transpose -> P^T (fp16)
  y^T = sum_k V_k^T-block @ P^T-block (PSUM f32) -> f32r
  out_partial = y^T.T @ Wproj-rows (accumulate 2 head-chunks) -> DMA out
"""
import numpy as np

B, T, C = 4, 2048, 2048
H, HD = 16, 128
N_CORES = 8
HPC = H // N_CORES          # heads per core = 2
SCALE = float(1.0 / np.sqrt(HD))
NEG = -1e9

# "fp16": all matmul operands fp16 (1 cyc/row on PE).
# "fp32r": x/W path in fp32r (TF32-like, 2 passes -> 2x slower, ~2.5x more accurate).
MM_DT = "fp16"

_CACHE = {}


def _build_nc():
    import concourse.bass as bass
    from concourse import bacc
    import concourse.tile as tile
    import concourse.mybir as mybir
    from concourse.masks import make_identity, make_causal_mask
    from contextlib import ExitStack

    f32 = mybir.dt.float32
    f32r = mybir.dt.float32r
    f16 = mybir.dt.float16
    wdt = f16 if MM_DT == "fp16" else f32r
    in_dt = f32 if MM_DT == "fp32r" else f16
    Exp = mybir.ActivationFunctionType.Exp
    AX = mybir.AxisListType.X

    nc = bacc.Bacc("TRN2", target_bir_lowering=False, debug=False,
                   enable_asserts=True, num_devices=N_CORES)

    # Inputs (per-core shards prepared on host)
    xT = nc.dram_tensor("xt", [C, B * T], in_dt, kind="ExternalInput").ap()
    wqkv = nc.dram_tensor("wqkv", [C, 6 * HD], in_dt, kind="ExternalInput").ap()
    wproj = nc.dram_tensor("wproj", [HPC * HD, C], in_dt, kind="ExternalInput").ap()
    out = nc.dram_tensor("out", [B * T, C], f32, kind="ExternalOutput").ap()

    # DRAM views: c-chunked weights
    wqkv_v = wqkv.rearrange("(cc p) (jj d) -> p cc jj d", p=128, d=HD)  # [128,16,6,128]
    wproj_v = wproj.rearrange("(jh p) c -> p jh c", p=128)              # [128,2,2048]

    NCC = C // 128        # 16 contraction chunks
    NTCH = T // 512       # 4 t-chunks per batch

    with tile.TileContext(nc) as tc, ExitStack() as ctx:
        const = ctx.enter_context(tc.tile_pool(name="const", bufs=1))
        wpool = ctx.enter_context(tc.tile_pool(name="w", bufs=1))
        xtp = ctx.enter_context(tc.tile_pool(name="xt", bufs=2))
        qkvp = ctx.enter_context(tc.tile_pool(name="qkv", bufs=2))
        
        dnp = ctx.enter_context(tc.tile_pool(name="dn", bufs=4))
        rp = ctx.enter_context(tc.tile_pool(name="r", bufs=2))
        ptp = ctx.enter_context(tc.tile_pool(name="pt", bufs=2))
        ytp = ctx.enter_context(tc.tile_pool(name="yt", bufs=2))
        op = ctx.enter_context(tc.tile_pool(name="o", bufs=6))
        psA = ctx.enter_context(tc.tile_pool(name="psA", bufs=3, space="PSUM"))
        psV = ctx.enter_context(tc.tile_pool(name="psV", bufs=2, space="PSUM"))
        psD = ctx.enter_context(tc.tile_pool(name="psD", bufs=2, space="PSUM"))
        psT = ctx.enter_context(tc.tile_pool(name="psT", bufs=1, space="PSUM"))

        ident_f = const.tile([128, 128], f32)
        make_identity(nc, ident_f)
        ident_h = const.tile([128, 128], f16)
        nc.scalar.copy(ident_h, ident_f)
        # transposed-orientation causal mask: keep (partition=k_rel) <= (free=q_rel)
        triT = const.tile([128, 128], f32)
        nc.gpsimd.memset(triT, 0.0)
        nc.gpsimd.affine_select(
            out=triT, in_=triT, compare_op=mybir.AluOpType.is_ge, fill=NEG,
            base=0, pattern=[[1, 128]], channel_multiplier=-1)
        ones_col = const.tile([128, 1], f16)
        nc.vector.memset(ones_col, 1.0)
        ones_row = const.tile([1, 128], f16)
        nc.vector.memset(ones_row, 1.0)

        w_sb = wpool.tile([128, NCC, 6, HD], wdt)
        nc.sync.dma_start(w_sb, wqkv_v if MM_DT == "fp16" else wqkv_v.bitcast(f32r))
        wp_sb = wpool.tile([128, 2, C], wdt)
        nc.sync.dma_start(wp_sb, wproj_v if MM_DT == "fp16" else wproj_v.bitcast(f32r))

        def emit_qkv_chunk(b, tch, qkv_tiles):
            qt, kt, vt, v = qkv_tiles
            t0 = b * T + tch * 512
            xt_t = xtp.tile([128, NCC, 512], wdt, tag="xt")
            for cc in range(NCC):
                nc.sync.dma_start(
                    xt_t[:, cc, :],
                    xT[cc * 128:(cc + 1) * 128, t0:t0 + 512] if MM_DT == "fp16"
                    else xT[cc * 128:(cc + 1) * 128, t0:t0 + 512].bitcast(f32r))
            for jj in range(6):  # q_h0, q_h1, k_h0, k_h1, v_h0, v_h1
                qk_ps = psA.tile([128, 512], f32, tag="psA")
                for cc in range(NCC):
                    nc.tensor.matmul(qk_ps, w_sb[:, cc, jj, :], xt_t[:, cc, :],
                                     start=(cc == 0), stop=(cc == NCC - 1))
                dst = (qt, qt, kt, kt, vt, vt)[jj]
                nc.scalar.copy(dst[:, jj % 2, tch * 512:(tch + 1) * 512], qk_ps)
            # transpose this chunk's V^T slice -> V [t, d]
            for hh in range(HPC):
                for tb in range(4):
                    tg = tch * 4 + tb
                    vp = psT.tile([128, 128], f16, tag="psT")
                    nc.tensor.transpose(
                        vp, vt[:, hh, tg * 128:(tg + 1) * 128], ident_h)
                    nc.vector.tensor_copy(v[:, tg, hh * HD:(hh + 1) * HD], vp)

        def emit_attn_unit(b, qg, h, qkv_tiles, yt):
            qt, kt, vt, v = qkv_tiles
            pt_sb = ptp.tile([128, T // 128, 512], f16, tag="pt")
            den_ps = psD.tile([1, 512], f32, tag="psD")
            yt_ps = psV.tile([128, 512], f32, tag="psV")
            nkb = 4 * qg + 4
            for kb in range(nkb):
                kk = kb - 4 * qg
                qs = max(0, kk) * 128
                st = psA.tile([128, 512], f32, tag="psA")
                nc.tensor.matmul(
                    st[:, qs:512], kt[:, h, kb * 128:(kb + 1) * 128],
                    qt[:, h, qg * 512 + qs:(qg + 1) * 512],
                    start=True, stop=True)
                if kk >= 0:
                    nc.vector.tensor_add(
                        st[:, qs:qs + 128], st[:, qs:qs + 128], triT)
                nc.scalar.activation(
                    pt_sb[:, kb, qs:512], st[:, qs:512], Exp, scale=SCALE)
                nc.tensor.matmul(
                    den_ps[0:1, qs:512], ones_col, pt_sb[:, kb, qs:512],
                    start=(kb == 0), stop=(kb == nkb - 1))
                nc.tensor.matmul(
                    yt_ps[:, qs:512], v[:, kb, h * HD:(h + 1) * HD],
                    pt_sb[:, kb, qs:512],
                    start=(kb == 0), stop=(kb == nkb - 1))
            rec_row = dnp.tile([1, 512], f32, tag="rec")
            nc.vector.reciprocal(rec_row, den_ps[0:1, :])
            rec16 = dnp.tile([1, 512], f16, tag="rec16")
            nc.scalar.copy(rec16, rec_row)
            r_ps = psD.tile([128, 512], f32, tag="psD")
            nc.tensor.matmul(r_ps, ones_row, rec16, start=True, stop=True)
            r_sb = rp.tile([128, 512], f32, tag="rsb")
            nc.vector.tensor_copy(r_sb, r_ps)
            nc.vector.tensor_mul(yt[:, h, :], yt_ps, r_sb)

        def emit_proj(b, qg, yt):
            for tt in range(4):
                for co in range(4):
                    o_ps = psA.tile([128, 512], f32, tag="psA")
                    for jh in range(HPC):
                        nc.tensor.matmul(
                            o_ps, yt[:, jh, tt * 128:(tt + 1) * 128],
                            wp_sb[:, jh, co * 512:(co + 1) * 512],
                            start=(jh == 0), stop=(jh == HPC - 1))
                    o_sb = op.tile([128, 512], f32, tag="osb")
                    nc.vector.tensor_copy(o_sb, o_ps)
                    r0 = b * T + qg * 512 + tt * 128
                    nc.sync.dma_start(
                        out[r0:r0 + 128, co * 512:(co + 1) * 512], o_sb)

        def alloc_qkv_tiles():
            qt = qkvp.tile([128, HPC, T], wdt, tag="qt")
            kt = qkvp.tile([128, HPC, T], wdt, tag="kt")
            vt = qkvp.tile([128, HPC, T], f16, tag="vt")
            v = qkvp.tile([128, T // 128, HPC * HD], f16, tag="v")
            return (qt, kt, vt, v)

        # Pipeline: QKV chunks of batch b+1 are interleaved into the
        # attention/proj stream of batch b so the PE array never idles
        # long enough for HAM to re-throttle it.
        tiles = alloc_qkv_tiles()
        for tch in range(NTCH):
            emit_qkv_chunk(0, tch, tiles)
        for b in range(B):
            nxt = alloc_qkv_tiles() if b + 1 < B else None
            for qg in range(4):
                if nxt is not None:
                    emit_qkv_chunk(b + 1, qg, nxt)
                yt = ytp.tile([128, HPC, 512], wdt, tag="yt")
                for h in range(HPC):
                    emit_attn_unit(b, qg, h, tiles, yt)
                emit_proj(b, qg, yt)
            tiles = nxt

    nc.compile()
    return nc


def _get_nc():
    if "nc" not in _CACHE:
        _CACHE["nc"] = _build_nc()
    return _CACHE["nc"]


def _make_in_maps(x2d, Wqkv, Wproj):
    hdt = np.float16 if MM_DT == "fp16" else np.float32
    xT = np.ascontiguousarray(x2d.T).astype(hdt)  # [C, B*T]
    in_maps = []
    for c in range(N_CORES):
        h0 = c * HPC
        cols = []
        for part in range(3):  # q, k, v blocks of Wqkv columns
            for h in range(HPC):
                j0 = part * C + (h0 + h) * HD
                cols.append(Wqkv[:, j0:j0 + HD])
        wq = np.ascontiguousarray(np.concatenate(cols, axis=1)).astype(hdt)
        wp = np.ascontiguousarray(Wproj[h0 * HD:(h0 + HPC) * HD, :]).astype(hdt)
        in_maps.append({"xt": xT, "wqkv": wq, "wproj": wp})
    return in_maps


def run_shards(in_maps, trace=False):
    from concourse.bass_utils import run_bass_kernel_spmd
    nc = _get_nc()
    last_err = None
    for _attempt in range(3):
        try:
            return run_bass_kernel_spmd(
                nc, in_maps, core_ids=list(range(N_CORES)), trace=trace)
        except Exception as e:  # transient NRT device errors — retry
            last_err = e
            if "UNAVAILABLE" not in str(e) and "UNRECOVERABLE" not in str(e):
                raise
    raise last_err


def kernel(x, Wqkv, Wproj):
    x = np.asarray(x, dtype=np.float32)
    Wqkv = np.asarray(Wqkv, dtype=np.float32)
    Wproj = np.asarray(Wproj, dtype=np.float32)
    x2d = np.ascontiguousarray(x.reshape(B * T, C))

    in_maps = _make_in_maps(x2d, Wqkv, Wproj)
    res = run_shards(in_maps)

    acc = res.results[0]["out"].astype(np.float64)
    for c in range(1, N_CORES):
        acc += res.results[c]["out"]
    return acc.reshape(B, T, C).astype(np.float32)

